# revision 1
# baseline (speedup 1.0000x reference)
"""Self-contained distributed AFGCN kernel for 8 TRN2 NeuronCores.

kernel(**inputs) takes the FULL unsharded inputs (as produced by the
problem's setup_inputs) and returns the FULL [100000] float32 output.

Pipeline per core (SPMD, one compiled graph):
  - full padded node-feature replica in DRAM (4 int16-indexed banks),
    per-edge dma_gather of source rows;
  - scatter-add via one-hot (DVE is_equal vs plane-offset iota) matmuls
    accumulated in PSUM, bf16 operands / fp32 accumulation;
  - dense GraphConv update in feature-transposed layout (PE + ACT + DVE);
  - AllGather rebuilds the replica between layers;
  - fc head -> per-core output shard, host concatenates.
"""
import numpy as np
import ml_dtypes
from contextlib import ExitStack
from dataclasses import dataclass


@dataclass
class Cfg:
    N: int = 100000          # real nodes
    D: int = 64
    L: int = 4
    NC: int = 8
    NSH: int = 12544         # nodes per core (NBLK*128)
    GB: int = 14             # dst blocks per super-group
    NBANK: int = 4
    PADM: int = 16           # group cap quantum (full-col matmuls + plane-masked one-hot)
    MAXI: int = 1024         # idxs per dma_gather call
    DCH: int = 448           # dense node-chunk cols

    @property
    def NP(self):
        return self.NC * self.NSH

    @property
    def NBLK(self):
        return self.NSH // 128

    @property
    def NG(self):
        return self.NBLK // self.GB

    @property
    def BANK(self):
        return self.NP // self.NBANK


FULL = Cfg()
SMALL = Cfg(N=2000, NSH=256, GB=2, DCH=128)  # NP=2048, NBLK=2, NG=1


def roundup(x, m):
    return -(-x // m) * m


def make_plan(src_g, dst_g, cfg):
    """SPMD-uniform skeleton + per-core gather/onehot data."""
    cN = cfg
    per = []
    sizes = np.zeros((cN.NC, cN.NBLK, cN.NBANK), np.int64)
    for c in range(cN.NC):
        lo, hi = c * cN.NSH, (c + 1) * cN.NSH
        m = (dst_g >= lo) & (dst_g < hi)
        src, dst = src_g[m], dst_g[m] - lo
        blk = dst // 128
        bank = src // cN.BANK
        d = {}
        for b in range(cN.NBLK):
            mb = blk == b
            sb, db, kb = src[mb], dst[mb], bank[mb]
            for k in range(cN.NBANK):
                mk = kb == k
                d[(b, k)] = (sb[mk], db[mk])
                sizes[c, b, k] = mk.sum()
        per.append(d)

    caps = np.maximum(roundup(sizes.max(axis=0), cN.PADM), cN.PADM)

    runs, chunks = [], []
    S = 0
    for g in range(cN.NG):
        for k in range(cN.NBANK):
            run_start = S
            for b in range(g * cN.GB, (g + 1) * cN.GB):
                grp_start = S
                S += int(caps[b, k])
                s0 = grp_start
                while s0 < S:
                    s1 = min(S, (s0 // 128 + 1) * 128)
                    chunks.append(dict(col=s0 // 128, p0=s0 % 128,
                                       p1=s1 - (s0 // 128) * 128,
                                       block=b, g=g, bank=k))
                    s0 = s1
            pad = -(S - run_start) % 128
            S += pad
            size = S - run_start
            calls = []
            off = 0
            while off < size:
                ni = min(cN.MAXI, size - off)
                calls.append((run_start + off, ni))
                off += ni
            runs.append(dict(g=g, bank=k, start=run_start, size=size,
                             calls=calls))
    # start/stop flags per (block, bank) group, in col order
    by_bb = {}
    for ch in chunks:
        by_bb.setdefault((ch["block"], ch["bank"]), []).append(ch)
    for _, lst in by_bb.items():
        lst.sort(key=lambda ch: (ch["col"], ch["p0"]))
        for i, ch in enumerate(lst):
            ch["start"] = i == 0
            ch["stop"] = i == len(lst) - 1
    skeleton = dict(caps=caps, runs=runs, chunks=chunks, S=S)

    per_core = []
    run_of = {(r["g"], r["bank"]): r for r in runs}
    for c in range(cN.NC):
        slots_src = np.zeros(S, np.int64)
        dstloc = np.full(S, -1.0, np.float32)
        for g in range(cN.NG):
            for k in range(cN.NBANK):
                run = run_of[(g, k)]
                pos = run["start"]
                for b in range(g * cN.GB, (g + 1) * cN.GB):
                    sb, db = per[c][(b, k)]
                    n = len(sb)
                    slots_src[pos : pos + n] = sb
                    slots_src[pos + n : pos + int(caps[b, k])] = k * cN.BANK
                    dstloc[pos : pos + n] = (db - b * 128).astype(np.float32) \
                        + 128.0 * (b % 16)
                    pos += int(caps[b, k])
                slots_src[pos : run["start"] + run["size"]] = k * cN.BANK
        idxflat = (slots_src % cN.BANK).astype(np.int16)
        idx16 = np.zeros((128, S // 16), np.int16)
        for run in runs:
            for (off, ni) in run["calls"]:
                seg = idxflat[off : off + ni]
                w = seg.reshape(ni // 16, 16).T
                idx16[:, off // 16 : (off + ni) // 16] = np.tile(w, (8, 1))
        dstloc_t = dstloc.reshape(S // 128, 128).T.copy()
        per_core.append(dict(idx16=idx16, dstloc=dstloc_t,
                             slots_src=slots_src, dstloc_flat=dstloc))
    return skeleton, per_core


def plan_forward_numpy(inputs, cfg, skeleton, per_core):
    x = np.asarray(inputs["x"], np.float32)
    W_rel = np.asarray(inputs["W_rel"], np.float32)
    b_rel = np.asarray(inputs["b_rel"], np.float32)
    W_root = np.asarray(inputs["W_root"], np.float32)
    fc_w = np.asarray(inputs["fc_w"], np.float32)
    fc_b = np.asarray(inputs["fc_b"], np.float32)
    hp = np.zeros((cfg.NP, cfg.D), np.float32)
    hp[: cfg.N] = x
    h0 = hp.copy()
    for l in range(cfg.L):
        agg = np.zeros((cfg.NP, cfg.D), np.float32)
        for c in range(cfg.NC):
            pc = per_core[c]
            msgs = hp[pc["slots_src"]]
            dl = pc["dstloc_flat"]
            for ch in skeleton["chunks"]:
                s0 = ch["col"] * 128
                s1 = s0 + 128
                pl = ch["block"] % 16
                onehot = dl[s0:s1, None] == (np.arange(128)[None, :] + 128.0 * pl)
                base = c * cfg.NSH + ch["block"] * 128
                agg[base : base + 128] += onehot.astype(np.float32).T @ msgs[s0:s1]
        z = agg @ W_rel[l] + b_rel[l] + hp @ W_root[l]
        hp = np.maximum(z, 0.0) + h0
    out = hp @ fc_w + fc_b
    return out[: cfg.N, 0]


def build_nc(cfg, skeleton, fc_b_val):
    import concourse.bass as bass
    import concourse.tile as tile
    from concourse import bacc, mybir

    STAGE = 5

    f32 = mybir.dt.float32
    bf16 = mybir.dt.bfloat16
    i16 = mybir.dt.int16
    cN = cfg
    S = skeleton["S"]
    runs, chunks = skeleton["runs"], skeleton["chunks"]
    run_of = {(r["g"], r["bank"]): r for r in runs}

    nc = bacc.Bacc("TRN2", target_bir_lowering=False, debug=False,
                   num_devices=cN.NC, num_swdge_queues=4)

    t_x0 = nc.dram_tensor("x0", [cN.NP, cN.D], f32, kind="ExternalInput")
    t_xT = nc.dram_tensor("xT", [cN.D, cN.NSH], f32, kind="ExternalInput")
    t_idx = nc.dram_tensor("idx", [128, S // 16], i16, kind="ExternalInput")
    t_dstloc = nc.dram_tensor("dstloc", [128, S // 128], f32, kind="ExternalInput")
    t_iota = nc.dram_tensor("iota", [128, 2048], f32, kind="ExternalInput")
    t_ident = nc.dram_tensor("ident", [128, 128], f32, kind="ExternalInput")
    t_Wr = nc.dram_tensor("Wr", [cN.L, cN.D, cN.D], bf16, kind="ExternalInput")
    t_Wo = nc.dram_tensor("Wo", [cN.L, cN.D, cN.D], bf16, kind="ExternalInput")
    t_br = nc.dram_tensor("br", [cN.L, cN.D], f32, kind="ExternalInput")
    t_fcw = nc.dram_tensor("fcw", [cN.D, 1], f32, kind="ExternalInput")
    t_fcb = nc.dram_tensor("fcb", [1, 1], f32, kind="ExternalInput")
    t_out = nc.dram_tensor("out", [1, cN.NSH], f32, kind="ExternalOutput")

    t_inb, t_outb = [], []
    for l in range(cN.L - 1):
        t_inb.append(nc.dram_tensor(f"inb{l}", [cN.NSH, cN.D], f32))
        t_outb.append(nc.dram_tensor(f"outb{l}", [cN.NP, cN.D], f32,
                                     addr_space="Shared"))

    GCOLS = cN.GB * 128

    with tile.TileContext(nc) as tc, ExitStack() as ctx:
        c_pool = ctx.enter_context(tc.tile_pool(name="const", bufs=1))
        h_pool = ctx.enter_context(tc.tile_pool(name="hbuf", bufs=1))
        msg_pool = ctx.enter_context(tc.tile_pool(name="msg", bufs=2))
        msgb_pool = ctx.enter_context(tc.tile_pool(name="msgb", bufs=2))
        idx_pool = ctx.enter_context(tc.tile_pool(name="idxp", bufs=3))
        oh_pool = ctx.enter_context(tc.tile_pool(name="oh", bufs=6))
        st_pool = ctx.enter_context(tc.tile_pool(name="stage", bufs=2))
        ep_pool = ctx.enter_context(tc.tile_pool(name="epi", bufs=2))
        h0_pool = ctx.enter_context(tc.tile_pool(name="h0sl", bufs=2))
        ps_agg = ctx.enter_context(tc.tile_pool(name="psagg", bufs=1, space="PSUM"))
        ps_z = ctx.enter_context(tc.tile_pool(name="psz", bufs=2, space="PSUM"))
        ps_t = ctx.enter_context(tc.tile_pool(name="pst", bufs=1, space="PSUM"))
        ps_fc = ctx.enter_context(tc.tile_pool(name="psfc", bufs=1, space="PSUM"))

        dstloc_t = c_pool.tile([128, S // 128], f32, tag="dstloc")
        nc.sync.dma_start(out=dstloc_t[:], in_=t_dstloc[:])
        iota_t = c_pool.tile([128, 2048], f32, tag="iota")
        nc.sync.dma_start(out=iota_t[:], in_=t_iota[:])
        ident_t = c_pool.tile([128, 128], f32, tag="ident")
        nc.sync.dma_start(out=ident_t[:], in_=t_ident[:])
        wr_t, wo_t, br_t = [], [], []
        for l in range(cN.L):
            w1 = c_pool.tile([cN.D, cN.D], bf16, tag=f"wr{l}")
            nc.sync.dma_start(out=w1[:], in_=t_Wr[l, :, :])
            wr_t.append(w1)
            w2 = c_pool.tile([cN.D, cN.D], bf16, tag=f"wo{l}")
            nc.sync.dma_start(out=w2[:], in_=t_Wo[l, :, :])
            wo_t.append(w2)
            bb = c_pool.tile([cN.D, 1], f32, tag=f"br{l}")
            nc.sync.dma_start(out=bb[:], in_=t_br[l, :, None])
            br_t.append(bb)
        fcw_t = c_pool.tile([cN.D, 1], f32, tag="fcw")
        nc.sync.dma_start(out=fcw_t[:], in_=t_fcw[:])
        fcb_t = c_pool.tile([1, 1], f32, tag="fcb")
        nc.sync.dma_start(out=fcb_t[:], in_=t_fcb[:])

        gq = [0]
        hT = h_pool.tile([cN.D, cN.NSH], f32, tag="hT")
        nc.sync.dma_start(out=hT[:], in_=t_xT[:])

        for l in range(cN.L):
            table = t_x0 if (l == 0 or STAGE < 5) else t_outb[l - 1]
            for g in range(cN.NG):
                agg_sb = st_pool.tile([cN.D, GCOLS], f32, tag="aggsb")
                for k in range(cN.NBANK):
                    agg_ps = ps_agg.tile([cN.D, GCOLS], f32, tag="agg")
                    run = run_of[(g, k)]
                    ncols = run["size"] // 128
                    rc0 = run["start"] // 128
                    mt = msg_pool.tile([128, ncols * cN.D], f32, tag="msg")
                    bank_ap = table[k * cN.BANK : (k + 1) * cN.BANK, :]
                    for (off, ni) in run["calls"]:
                        it = idx_pool.tile([128, ni // 16], i16, tag="idxt")
                        nc.sync.dma_start(
                            out=it[:],
                            in_=t_idx[:, off // 16 : (off + ni) // 16])
                        o0 = (off - run["start"]) // 128
                        nc.gpsimd.dma_gather(
                            out_ap=mt[:, o0 * cN.D : (o0 + ni // 128) * cN.D]
                                .rearrange("p (c d) -> p c d", d=cN.D),
                            in_ap=bank_ap,
                            idxs_ap=it[:],
                            num_idxs=ni,
                            num_idxs_reg=ni,
                            elem_size=cN.D,
                            queue_num=gq[0] % 4,
                        )
                        gq[0] += 1
                    if STAGE < 2:
                        continue
                    mtb = msgb_pool.tile([128, ncols * cN.D], bf16, tag="msgb")
                    nc.vector.tensor_copy(out=mtb[:], in_=mt[:])
                    for ch in [c for c in chunks
                               if STAGE >= 3 and c["g"] == g and c["bank"] == k]:
                        rel = ch["col"] - rc0
                        bcol = (ch["block"] - g * cN.GB) * 128
                        pl = ch["block"] % 16
                        oh = oh_pool.tile([128, 128], bf16, tag="oh")
                        nc.vector.tensor_tensor(
                            out=oh[:],
                            in0=dstloc_t[:, ch["col"] : ch["col"] + 1]
                                .to_broadcast([128, 128]),
                            in1=iota_t[:, pl * 128 : (pl + 1) * 128],
                            op=mybir.AluOpType.is_equal,
                        )
                        nc.tensor.matmul(
                            out=agg_ps[:, bcol : bcol + 128],
                            lhsT=mtb[:, rel * cN.D : (rel + 1) * cN.D],
                            rhs=oh[:],
                            start=ch["start"],
                            stop=ch["stop"],
                        )
                    if STAGE >= 3:
                        if k == 0:
                            nc.vector.tensor_copy(out=agg_sb[:], in_=agg_ps[:])
                        else:
                            nc.vector.tensor_add(out=agg_sb[:], in0=agg_sb[:],
                                                 in1=agg_ps[:])
                gb0 = g * GCOLS
                if STAGE >= 4:
                    h0sl = h0_pool.tile([cN.D, GCOLS], f32, tag="h0sl")
                    nc.sync.dma_start(out=h0sl[:],
                                      in_=t_xT[:, gb0 : gb0 + GCOLS])
                for j in range(GCOLS // cN.DCH if STAGE >= 4 else 0):
                    cl = gb0 + j * cN.DCH
                    aggb = ep_pool.tile([cN.D, cN.DCH], bf16, tag="aggb")
                    nc.vector.tensor_copy(
                        out=aggb[:], in_=agg_sb[:, j * cN.DCH : (j + 1) * cN.DCH])
                    hTb = ep_pool.tile([cN.D, cN.DCH], bf16, tag="hTb")
                    nc.vector.tensor_copy(out=hTb[:], in_=hT[:, cl : cl + cN.DCH])
                    zps = ps_z.tile([cN.D, cN.DCH], f32, tag="z")
                    nc.tensor.matmul(out=zps[:], lhsT=wr_t[l][:],
                                     rhs=aggb[:], start=True, stop=False)
                    nc.tensor.matmul(out=zps[:], lhsT=wo_t[l][:],
                                     rhs=hTb[:], start=False, stop=True)
                    ep = ep_pool.tile([cN.D, cN.DCH], f32, tag="ep")
                    nc.scalar.activation(
                        out=ep[:], in_=zps[:],
                        func=mybir.ActivationFunctionType.Relu,
                        bias=br_t[l][:], scale=1.0)
                    nc.vector.tensor_add(out=hT[:, cl : cl + cN.DCH],
                                         in0=ep[:],
                                         in1=h0sl[:, j * cN.DCH : (j + 1) * cN.DCH])
                if l < cN.L - 1 and STAGE >= 5:
                    row_sb = st_pool.tile([128, cN.GB * cN.D], f32, tag="rows")
                    for b in range(cN.GB):
                        tps = ps_t.tile([128, cN.D], f32, tag="tps")
                        nc.tensor.transpose(
                            out=tps[:],
                            in_=hT[:, gb0 + b * 128 : gb0 + (b + 1) * 128],
                            identity=ident_t[: cN.D, : cN.D])
                        nc.vector.tensor_copy(
                            out=row_sb[:, b * cN.D : (b + 1) * cN.D],
                            in_=tps[:])
                    nc.sync.dma_start(
                        out=t_inb[l][gb0 : gb0 + GCOLS, :]
                            .rearrange("(k p) d -> p k d", p=128),
                        in_=row_sb[:].rearrange("p (k d) -> p k d", d=cN.D))
            if l < cN.L - 1 and STAGE >= 5:
                nc.gpsimd.collective_compute(
                    "AllGather",
                    mybir.AluOpType.bypass,
                    ins=[t_inb[l][:]],
                    outs=[t_outb[l][:]],
                    replica_groups=[list(range(cN.NC))],
                )
        for j in range(cN.NSH // cN.DCH):
            fps = ps_fc.tile([1, cN.DCH], f32, tag="fc")
            nc.tensor.matmul(out=fps[:], lhsT=fcw_t[:],
                             rhs=hT[:, j * cN.DCH : (j + 1) * cN.DCH],
                             start=True, stop=True)
            osb = ep_pool.tile([1, cN.DCH], f32, tag="osb")
            nc.scalar.activation(
                out=osb[:], in_=fps[:],
                func=mybir.ActivationFunctionType.Identity,
                bias=fcb_t[:], scale=1.0)
            nc.sync.dma_start(
                out=t_out[:, j * cN.DCH : (j + 1) * cN.DCH], in_=osb[:])

    nc.compile()
    return nc


def make_in_maps(inputs, cfg, per_core):
    x = np.asarray(inputs["x"], np.float32)
    xp = np.zeros((cfg.NP, cfg.D), np.float32)
    xp[: cfg.N] = x
    iota = np.tile(
        np.concatenate([np.arange(128, dtype=np.float32) + 128.0 * p
                        for p in range(16)])[None, :], (128, 1))
    ident = np.eye(128, dtype=np.float32)
    in_maps = []
    for c in range(cfg.NC):
        xT = np.ascontiguousarray(xp[c * cfg.NSH : (c + 1) * cfg.NSH].T)
        in_maps.append(dict(
            x0=xp, xT=xT,
            idx=per_core[c]["idx16"], dstloc=per_core[c]["dstloc"],
            iota=iota, ident=ident,
            Wr=np.ascontiguousarray(np.asarray(inputs["W_rel"], np.float32)).astype(ml_dtypes.bfloat16),
            Wo=np.ascontiguousarray(np.asarray(inputs["W_root"], np.float32)).astype(ml_dtypes.bfloat16),
            br=np.ascontiguousarray(np.asarray(inputs["b_rel"], np.float32)),
            fcw=np.ascontiguousarray(np.asarray(inputs["fc_w"], np.float32)),
            fcb=np.asarray(inputs["fc_b"], np.float32).reshape(1, 1),
        ))
    return in_maps


def run(inputs, cfg, trace=True):
    from concourse.bass_utils import run_bass_kernel_spmd

    src_g = np.asarray(inputs["edge_index"][0]).astype(np.int64)
    dst_g = np.asarray(inputs["edge_index"][1]).astype(np.int64)
    skeleton, per_core = make_plan(src_g, dst_g, cfg)
    fc_b_val = float(np.asarray(inputs["fc_b"]).ravel()[0])
    nc = build_nc(cfg, skeleton, fc_b_val)
    in_maps = make_in_maps(inputs, cfg, per_core)
    res = run_bass_kernel_spmd(nc, in_maps, list(range(cfg.NC)), trace=trace)
    outs = [np.asarray(res.results[c]["out"]).ravel() for c in range(cfg.NC)]
    full = np.concatenate(outs)[: cfg.N]
    return full, res


def kernel(**inputs):
    """Full inputs -> full output [N] float32."""
    cfg = FULL
    src_g = np.asarray(inputs["edge_index"][0]).astype(np.int64)
    dst_g = np.asarray(inputs["edge_index"][1]).astype(np.int64)
    skeleton, per_core = make_plan(src_g, dst_g, cfg)
    fc_b_val = float(np.asarray(inputs["fc_b"]).ravel()[0])
    nc = build_nc(cfg, skeleton, fc_b_val)
    in_maps = make_in_maps(inputs, cfg, per_core)
    from concourse.bass_utils import run_bass_kernel_spmd
    res = run_bass_kernel_spmd(nc, in_maps, list(range(cfg.NC)), trace=False)
    outs = [np.asarray(res.results[c]["out"]).ravel() for c in range(cfg.NC)]
    return np.concatenate(outs)[: cfg.N].astype(np.float32)



# revision 4
# speedup vs baseline: 1.0091x; 1.0091x over previous
"""Self-contained distributed AFGCN kernel for 8 TRN2 NeuronCores.

kernel(**inputs) takes the FULL unsharded inputs (as produced by the
problem's setup_inputs) and returns the FULL [100000] float32 output.

Pipeline per core (SPMD, one compiled graph):
  - full padded node-feature replica in DRAM (4 int16-indexed banks),
    per-edge dma_gather of source rows;
  - scatter-add via one-hot (DVE is_equal vs plane-offset iota) matmuls
    accumulated in PSUM, bf16 operands / fp32 accumulation;
  - dense GraphConv update in feature-transposed layout (PE + ACT + DVE);
  - AllGather rebuilds the replica between layers;
  - fc head -> per-core output shard, host concatenates.
"""
import numpy as np
import ml_dtypes
from contextlib import ExitStack
from dataclasses import dataclass


@dataclass
class Cfg:
    N: int = 100000          # real nodes
    D: int = 64
    L: int = 4
    NC: int = 8
    NSH: int = 12544         # nodes per core (NBLK*128)
    GB: int = 14             # dst blocks per super-group
    NBANK: int = 4
    PADM: int = 16           # group cap quantum (full-col matmuls + plane-masked one-hot)
    MAXI: int = 1024         # idxs per dma_gather call
    DCH: int = 448           # dense node-chunk cols

    @property
    def NP(self):
        return self.NC * self.NSH

    @property
    def NBLK(self):
        return self.NSH // 128

    @property
    def NG(self):
        return self.NBLK // self.GB

    @property
    def BANK(self):
        return self.NP // self.NBANK


FULL = Cfg()
SMALL = Cfg(N=2000, NSH=256, GB=2, DCH=128)  # NP=2048, NBLK=2, NG=1


def roundup(x, m):
    return -(-x // m) * m


def make_plan(src_g, dst_g, cfg):
    """SPMD-uniform skeleton + per-core gather/onehot data."""
    cN = cfg
    per = []
    sizes = np.zeros((cN.NC, cN.NBLK, cN.NBANK), np.int64)
    for c in range(cN.NC):
        lo, hi = c * cN.NSH, (c + 1) * cN.NSH
        m = (dst_g >= lo) & (dst_g < hi)
        src, dst = src_g[m], dst_g[m] - lo
        blk = dst // 128
        bank = src // cN.BANK
        d = {}
        for b in range(cN.NBLK):
            mb = blk == b
            sb, db, kb = src[mb], dst[mb], bank[mb]
            for k in range(cN.NBANK):
                mk = kb == k
                d[(b, k)] = (sb[mk], db[mk])
                sizes[c, b, k] = mk.sum()
        per.append(d)

    caps = np.maximum(roundup(sizes.max(axis=0), cN.PADM), cN.PADM)

    runs, chunks = [], []
    S = 0
    for g in range(cN.NG):
        for k in range(cN.NBANK):
            run_start = S
            for b in range(g * cN.GB, (g + 1) * cN.GB):
                grp_start = S
                S += int(caps[b, k])
                s0 = grp_start
                while s0 < S:
                    s1 = min(S, (s0 // 128 + 1) * 128)
                    chunks.append(dict(col=s0 // 128, p0=s0 % 128,
                                       p1=s1 - (s0 // 128) * 128,
                                       block=b, g=g, bank=k))
                    s0 = s1
            pad = -(S - run_start) % 128
            S += pad
            size = S - run_start
            calls = []
            off = 0
            while off < size:
                ni = min(cN.MAXI, size - off)
                calls.append((run_start + off, ni))
                off += ni
            runs.append(dict(g=g, bank=k, start=run_start, size=size,
                             calls=calls))
    # start/stop flags per (block, bank) group, in col order
    by_bb = {}
    for ch in chunks:
        by_bb.setdefault((ch["block"], ch["bank"]), []).append(ch)
    for _, lst in by_bb.items():
        lst.sort(key=lambda ch: (ch["col"], ch["p0"]))
        for i, ch in enumerate(lst):
            ch["start"] = i == 0
            ch["stop"] = i == len(lst) - 1
    skeleton = dict(caps=caps, runs=runs, chunks=chunks, S=S)

    per_core = []
    run_of = {(r["g"], r["bank"]): r for r in runs}
    for c in range(cN.NC):
        slots_src = np.zeros(S, np.int64)
        dstloc = np.full(S, -1.0, np.float32)
        for g in range(cN.NG):
            for k in range(cN.NBANK):
                run = run_of[(g, k)]
                pos = run["start"]
                for b in range(g * cN.GB, (g + 1) * cN.GB):
                    sb, db = per[c][(b, k)]
                    n = len(sb)
                    slots_src[pos : pos + n] = sb
                    slots_src[pos + n : pos + int(caps[b, k])] = k * cN.BANK
                    dstloc[pos : pos + n] = (db - b * 128).astype(np.float32) \
                        + 128.0 * (b % 16)
                    pos += int(caps[b, k])
                slots_src[pos : run["start"] + run["size"]] = k * cN.BANK
        idxflat = (slots_src % cN.BANK).astype(np.int16)
        idx16 = np.zeros((128, S // 16), np.int16)
        for run in runs:
            for (off, ni) in run["calls"]:
                seg = idxflat[off : off + ni]
                w = seg.reshape(ni // 16, 16).T
                idx16[:, off // 16 : (off + ni) // 16] = np.tile(w, (8, 1))
        dstloc_t = dstloc.reshape(S // 128, 128).T.copy()
        per_core.append(dict(idx16=idx16, dstloc=dstloc_t,
                             slots_src=slots_src, dstloc_flat=dstloc))
    return skeleton, per_core


def plan_forward_numpy(inputs, cfg, skeleton, per_core):
    x = np.asarray(inputs["x"], np.float32)
    W_rel = np.asarray(inputs["W_rel"], np.float32)
    b_rel = np.asarray(inputs["b_rel"], np.float32)
    W_root = np.asarray(inputs["W_root"], np.float32)
    fc_w = np.asarray(inputs["fc_w"], np.float32)
    fc_b = np.asarray(inputs["fc_b"], np.float32)
    hp = np.zeros((cfg.NP, cfg.D), np.float32)
    hp[: cfg.N] = x
    h0 = hp.copy()
    for l in range(cfg.L):
        agg = np.zeros((cfg.NP, cfg.D), np.float32)
        for c in range(cfg.NC):
            pc = per_core[c]
            msgs = hp[pc["slots_src"]]
            dl = pc["dstloc_flat"]
            for ch in skeleton["chunks"]:
                s0 = ch["col"] * 128
                s1 = s0 + 128
                pl = ch["block"] % 16
                onehot = dl[s0:s1, None] == (np.arange(128)[None, :] + 128.0 * pl)
                base = c * cfg.NSH + ch["block"] * 128
                agg[base : base + 128] += onehot.astype(np.float32).T @ msgs[s0:s1]
        z = agg @ W_rel[l] + b_rel[l] + hp @ W_root[l]
        hp = np.maximum(z, 0.0) + h0
    out = hp @ fc_w + fc_b
    return out[: cfg.N, 0]


def build_nc(cfg, skeleton, fc_b_val):
    import concourse.bass as bass
    import concourse.tile as tile
    from concourse import bacc, mybir

    STAGE = 5

    f32 = mybir.dt.float32
    bf16 = mybir.dt.bfloat16
    i16 = mybir.dt.int16
    cN = cfg
    S = skeleton["S"]
    runs, chunks = skeleton["runs"], skeleton["chunks"]
    run_of = {(r["g"], r["bank"]): r for r in runs}

    nc = bacc.Bacc("TRN2", target_bir_lowering=False, debug=False,
                   num_devices=cN.NC, num_swdge_queues=4,
                   dynamic_dma_scratch_size=65536)

    t_x0 = nc.dram_tensor("x0", [cN.NP, cN.D], f32, kind="ExternalInput")
    t_xT = nc.dram_tensor("xT", [cN.D, cN.NSH], f32, kind="ExternalInput")
    t_idx = nc.dram_tensor("idx", [128, S // 16], i16, kind="ExternalInput")
    t_dstloc = nc.dram_tensor("dstloc", [128, S // 128], f32, kind="ExternalInput")
    t_iota = nc.dram_tensor("iota", [128, 2048], f32, kind="ExternalInput")
    t_ident = nc.dram_tensor("ident", [128, 128], f32, kind="ExternalInput")
    t_Wr = nc.dram_tensor("Wr", [cN.L, cN.D, cN.D], bf16, kind="ExternalInput")
    t_Wo = nc.dram_tensor("Wo", [cN.L, cN.D, cN.D], bf16, kind="ExternalInput")
    t_br = nc.dram_tensor("br", [cN.L, cN.D], f32, kind="ExternalInput")
    t_fcw = nc.dram_tensor("fcw", [cN.D, 1], f32, kind="ExternalInput")
    t_fcb = nc.dram_tensor("fcb", [1, 1], f32, kind="ExternalInput")
    t_out = nc.dram_tensor("out", [1, cN.NSH], f32, kind="ExternalOutput")

    t_inb, t_outb = [], []
    for l in range(cN.L - 1):
        t_inb.append(nc.dram_tensor(f"inb{l}", [cN.NSH, cN.D], f32))
        t_outb.append(nc.dram_tensor(f"outb{l}", [cN.NP, cN.D], f32,
                                     addr_space="Shared"))

    GCOLS = cN.GB * 128

    with tile.TileContext(nc) as tc, ExitStack() as ctx:
        c_pool = ctx.enter_context(tc.tile_pool(name="const", bufs=1))
        h_pool = ctx.enter_context(tc.tile_pool(name="hbuf", bufs=1))
        msg_pool = ctx.enter_context(tc.tile_pool(name="msg", bufs=2))
        msgb_pool = ctx.enter_context(tc.tile_pool(name="msgb", bufs=2))
        idx_pool = ctx.enter_context(tc.tile_pool(name="idxp", bufs=3))
        oh_pool = ctx.enter_context(tc.tile_pool(name="oh", bufs=6))
        st_pool = ctx.enter_context(tc.tile_pool(name="stage", bufs=2))
        ep_pool = ctx.enter_context(tc.tile_pool(name="epi", bufs=2))
        h0_pool = ctx.enter_context(tc.tile_pool(name="h0sl", bufs=2))
        ps_agg = ctx.enter_context(tc.tile_pool(name="psagg", bufs=1, space="PSUM"))
        ps_z = ctx.enter_context(tc.tile_pool(name="psz", bufs=2, space="PSUM"))
        ps_t = ctx.enter_context(tc.tile_pool(name="pst", bufs=1, space="PSUM"))
        ps_fc = ctx.enter_context(tc.tile_pool(name="psfc", bufs=1, space="PSUM"))

        dstloc_t = c_pool.tile([128, S // 128], f32, tag="dstloc")
        nc.sync.dma_start(out=dstloc_t[:], in_=t_dstloc[:])
        iota_t = c_pool.tile([128, 2048], f32, tag="iota")
        nc.sync.dma_start(out=iota_t[:], in_=t_iota[:])
        ident_t = c_pool.tile([128, 128], f32, tag="ident")
        nc.sync.dma_start(out=ident_t[:], in_=t_ident[:])
        wr_t, wo_t, br_t = [], [], []
        for l in range(cN.L):
            w1 = c_pool.tile([cN.D, cN.D], bf16, tag=f"wr{l}")
            nc.sync.dma_start(out=w1[:], in_=t_Wr[l, :, :])
            wr_t.append(w1)
            w2 = c_pool.tile([cN.D, cN.D], bf16, tag=f"wo{l}")
            nc.sync.dma_start(out=w2[:], in_=t_Wo[l, :, :])
            wo_t.append(w2)
            bb = c_pool.tile([cN.D, 1], f32, tag=f"br{l}")
            nc.sync.dma_start(out=bb[:], in_=t_br[l, :, None])
            br_t.append(bb)
        fcw_t = c_pool.tile([cN.D, 1], f32, tag="fcw")
        nc.sync.dma_start(out=fcw_t[:], in_=t_fcw[:])
        fcb_t = c_pool.tile([1, 1], f32, tag="fcb")
        nc.sync.dma_start(out=fcb_t[:], in_=t_fcb[:])

        gq = [0]
        hT = h_pool.tile([cN.D, cN.NSH], f32, tag="hT")
        nc.sync.dma_start(out=hT[:], in_=t_xT[:])

        for l in range(cN.L):
            table = t_x0 if (l == 0 or STAGE < 5) else t_outb[l - 1]
            for g in range(cN.NG):
                agg_sb = st_pool.tile([cN.D, GCOLS], f32, tag="aggsb")
                for k in range(cN.NBANK):
                    agg_ps = ps_agg.tile([cN.D, GCOLS], f32, tag="agg")
                    run = run_of[(g, k)]
                    ncols = run["size"] // 128
                    rc0 = run["start"] // 128
                    mt = msg_pool.tile([128, ncols * cN.D], f32, tag="msg")
                    bank_ap = table[k * cN.BANK : (k + 1) * cN.BANK, :]
                    for (off, ni) in run["calls"]:
                        it = idx_pool.tile([128, ni // 16], i16, tag="idxt")
                        nc.sync.dma_start(
                            out=it[:],
                            in_=t_idx[:, off // 16 : (off + ni) // 16])
                        o0 = (off - run["start"]) // 128
                        nc.gpsimd.dma_gather(
                            out_ap=mt[:, o0 * cN.D : (o0 + ni // 128) * cN.D]
                                .rearrange("p (c d) -> p c d", d=cN.D),
                            in_ap=bank_ap,
                            idxs_ap=it[:],
                            num_idxs=ni,
                            num_idxs_reg=ni,
                            elem_size=cN.D,
                            queue_num=gq[0] % 4,
                        )
                        gq[0] += 1
                    if STAGE < 2:
                        continue
                    mtb = msgb_pool.tile([128, ncols * cN.D], bf16, tag="msgb")
                    nc.vector.tensor_copy(out=mtb[:], in_=mt[:])
                    for ch in [c for c in chunks
                               if STAGE >= 3 and c["g"] == g and c["bank"] == k]:
                        rel = ch["col"] - rc0
                        bcol = (ch["block"] - g * cN.GB) * 128
                        pl = ch["block"] % 16
                        oh = oh_pool.tile([128, 128], bf16, tag="oh")
                        nc.vector.tensor_tensor(
                            out=oh[:],
                            in0=dstloc_t[:, ch["col"] : ch["col"] + 1]
                                .to_broadcast([128, 128]),
                            in1=iota_t[:, pl * 128 : (pl + 1) * 128],
                            op=mybir.AluOpType.is_equal,
                        )
                        nc.tensor.matmul(
                            out=agg_ps[:, bcol : bcol + 128],
                            lhsT=mtb[:, rel * cN.D : (rel + 1) * cN.D],
                            rhs=oh[:],
                            start=ch["start"],
                            stop=ch["stop"],
                        )
                    if STAGE >= 3:
                        if k == 0:
                            nc.vector.tensor_copy(out=agg_sb[:], in_=agg_ps[:])
                        else:
                            nc.vector.tensor_add(out=agg_sb[:], in0=agg_sb[:],
                                                 in1=agg_ps[:])
                gb0 = g * GCOLS
                if STAGE >= 4:
                    h0sl = h0_pool.tile([cN.D, GCOLS], f32, tag="h0sl")
                    nc.sync.dma_start(out=h0sl[:],
                                      in_=t_xT[:, gb0 : gb0 + GCOLS])
                for j in range(GCOLS // cN.DCH if STAGE >= 4 else 0):
                    cl = gb0 + j * cN.DCH
                    aggb = ep_pool.tile([cN.D, cN.DCH], bf16, tag="aggb")
                    nc.vector.tensor_copy(
                        out=aggb[:], in_=agg_sb[:, j * cN.DCH : (j + 1) * cN.DCH])
                    hTb = ep_pool.tile([cN.D, cN.DCH], bf16, tag="hTb")
                    nc.vector.tensor_copy(out=hTb[:], in_=hT[:, cl : cl + cN.DCH])
                    zps = ps_z.tile([cN.D, cN.DCH], f32, tag="z")
                    nc.tensor.matmul(out=zps[:], lhsT=wr_t[l][:],
                                     rhs=aggb[:], start=True, stop=False)
                    nc.tensor.matmul(out=zps[:], lhsT=wo_t[l][:],
                                     rhs=hTb[:], start=False, stop=True)
                    ep = ep_pool.tile([cN.D, cN.DCH], f32, tag="ep")
                    nc.scalar.activation(
                        out=ep[:], in_=zps[:],
                        func=mybir.ActivationFunctionType.Relu,
                        bias=br_t[l][:], scale=1.0)
                    nc.vector.tensor_add(out=hT[:, cl : cl + cN.DCH],
                                         in0=ep[:],
                                         in1=h0sl[:, j * cN.DCH : (j + 1) * cN.DCH])
                if l < cN.L - 1 and STAGE >= 5:
                    row_sb = st_pool.tile([128, cN.GB * cN.D], f32, tag="rows")
                    for b in range(cN.GB):
                        tps = ps_t.tile([128, cN.D], f32, tag="tps")
                        nc.tensor.transpose(
                            out=tps[:],
                            in_=hT[:, gb0 + b * 128 : gb0 + (b + 1) * 128],
                            identity=ident_t[: cN.D, : cN.D])
                        nc.vector.tensor_copy(
                            out=row_sb[:, b * cN.D : (b + 1) * cN.D],
                            in_=tps[:])
                    nc.sync.dma_start(
                        out=t_inb[l][gb0 : gb0 + GCOLS, :]
                            .rearrange("(k p) d -> p k d", p=128),
                        in_=row_sb[:].rearrange("p (k d) -> p k d", d=cN.D))
            if l < cN.L - 1 and STAGE >= 5:
                nc.gpsimd.collective_compute(
                    "AllGather",
                    mybir.AluOpType.bypass,
                    ins=[t_inb[l][:]],
                    outs=[t_outb[l][:]],
                    replica_groups=[list(range(cN.NC))],
                )
        for j in range(cN.NSH // cN.DCH):
            fps = ps_fc.tile([1, cN.DCH], f32, tag="fc")
            nc.tensor.matmul(out=fps[:], lhsT=fcw_t[:],
                             rhs=hT[:, j * cN.DCH : (j + 1) * cN.DCH],
                             start=True, stop=True)
            osb = ep_pool.tile([1, cN.DCH], f32, tag="osb")
            nc.scalar.activation(
                out=osb[:], in_=fps[:],
                func=mybir.ActivationFunctionType.Identity,
                bias=fcb_t[:], scale=1.0)
            nc.sync.dma_start(
                out=t_out[:, j * cN.DCH : (j + 1) * cN.DCH], in_=osb[:])

    nc.compile()
    return nc


def make_in_maps(inputs, cfg, per_core):
    x = np.asarray(inputs["x"], np.float32)
    xp = np.zeros((cfg.NP, cfg.D), np.float32)
    xp[: cfg.N] = x
    iota = np.tile(
        np.concatenate([np.arange(128, dtype=np.float32) + 128.0 * p
                        for p in range(16)])[None, :], (128, 1))
    ident = np.eye(128, dtype=np.float32)
    in_maps = []
    for c in range(cfg.NC):
        xT = np.ascontiguousarray(xp[c * cfg.NSH : (c + 1) * cfg.NSH].T)
        in_maps.append(dict(
            x0=xp, xT=xT,
            idx=per_core[c]["idx16"], dstloc=per_core[c]["dstloc"],
            iota=iota, ident=ident,
            Wr=np.ascontiguousarray(np.asarray(inputs["W_rel"], np.float32)).astype(ml_dtypes.bfloat16),
            Wo=np.ascontiguousarray(np.asarray(inputs["W_root"], np.float32)).astype(ml_dtypes.bfloat16),
            br=np.ascontiguousarray(np.asarray(inputs["b_rel"], np.float32)),
            fcw=np.ascontiguousarray(np.asarray(inputs["fc_w"], np.float32)),
            fcb=np.asarray(inputs["fc_b"], np.float32).reshape(1, 1),
        ))
    return in_maps


def run(inputs, cfg, trace=True):
    from concourse.bass_utils import run_bass_kernel_spmd

    src_g = np.asarray(inputs["edge_index"][0]).astype(np.int64)
    dst_g = np.asarray(inputs["edge_index"][1]).astype(np.int64)
    skeleton, per_core = make_plan(src_g, dst_g, cfg)
    fc_b_val = float(np.asarray(inputs["fc_b"]).ravel()[0])
    nc = build_nc(cfg, skeleton, fc_b_val)
    in_maps = make_in_maps(inputs, cfg, per_core)
    res = run_bass_kernel_spmd(nc, in_maps, list(range(cfg.NC)), trace=trace)
    outs = [np.asarray(res.results[c]["out"]).ravel() for c in range(cfg.NC)]
    full = np.concatenate(outs)[: cfg.N]
    return full, res


def kernel(**inputs):
    """Full inputs -> full output [N] float32."""
    cfg = FULL
    src_g = np.asarray(inputs["edge_index"][0]).astype(np.int64)
    dst_g = np.asarray(inputs["edge_index"][1]).astype(np.int64)
    skeleton, per_core = make_plan(src_g, dst_g, cfg)
    fc_b_val = float(np.asarray(inputs["fc_b"]).ravel()[0])
    nc = build_nc(cfg, skeleton, fc_b_val)
    in_maps = make_in_maps(inputs, cfg, per_core)
    from concourse.bass_utils import run_bass_kernel_spmd
    res = run_bass_kernel_spmd(nc, in_maps, list(range(cfg.NC)), trace=False)
    outs = [np.asarray(res.results[c]["out"]).ravel() for c in range(cfg.NC)]
    return np.concatenate(outs)[: cfg.N].astype(np.float32)

